# revision 1
# baseline (speedup 1.0000x reference)
"""DBRX MoE experts kernel for Trainium2 (8 NeuronCores).

Strategy (balanced expert-parallel, all-bf16, zero collectives):
  - Router (logits -> softmax -> top-2 -> renormalize) computed on host in
    numpy (0.01% of FLOPs); it determines the token->expert dispatch.
  - Every core runs the identical SPMD program over S = s1 + s2 token slots,
    two segments with independent weight sets (per-core input data):
      seg1 (s1 slots): core c's own expert c, tokens [0, min(cnt_c, s1)).
      seg2 (s2 slots): an overflow chunk of some (possibly other) expert's
        tokens beyond s1, assigned per-core so the 8 chunks cover all
        overflow. (s1, s2) minimizes s1+s2 subject to the overflow of every
        expert packing into <= 8 chunks of size s2 -- near-perfect balance
        (e.g. 1036 slots vs a 1063-token max expert) at the cost of
        streaming a second expert's weights (hidden under compute).
  - Everything is bf16 (weights, tokens, h, output); PSUM accumulates fp32.
    bf16 keeps the tensor engine at 1 cycle/row for any moving-dim size and
    halves HBM traffic. Output rows are the *unweighted* FFN outputs; the
    host scales by the renormalized top-2 weights and sums (fp32).
  - Phase A (gate/up): per 128-row I-chunk, stream gate/up weight halves
    once; tokens are the moving dim. h = silu(gate)*up stays resident in
    SBUF (bf16). Phase B (down): per 128-row D-chunk, stream w2; accumulate
    over all 32 I-chunks into one PSUM bank; evacuate bf16 and DMA out.
  - Token DMAs are chunk-major ([dc, tok] contiguous per block) so every
    chunk moves at full DMA bandwidth regardless of size; startup chunks are
    small so the first matmul fires ~6.5us in, and a PE warm-up chain of
    dummy matmuls covers the DMA preamble (the PE clock ramps from half
    speed over 3us after any idle).
"""

import numpy as np

T = 4096
D = 2048
E = 8
I = 4096
NCORES = 8
P = 128
DCH = D // P  # 16 d-chunks
ICH = I // P  # 32 i-chunks
BLK = 512  # token block (PSUM bank = 512 fp32)


def _host_router(x, router_w):
    """Replicate reference routing in numpy (fp32)."""
    logits = (x.astype(np.float64) @ router_w.astype(np.float64).T).astype(np.float32)
    m = logits.max(axis=-1, keepdims=True)
    ex = np.exp((logits - m).astype(np.float32))
    probs = ex / ex.sum(axis=-1, keepdims=True)
    # top-2, ties to lower index (matches jax.lax.top_k)
    top1 = probs.argmax(axis=-1)
    p = probs.copy()
    p[np.arange(T), top1] = -1.0
    top2 = p.argmax(axis=-1)
    w1 = probs[np.arange(T), top1]
    w2 = probs[np.arange(T), top2]
    s = w1 + w2
    return top1.astype(np.int64), top2.astype(np.int64), (w1 / s).astype(np.float32), (w2 / s).astype(np.float32)


def _split_slots(cnts):
    """Minimize S = s1 + s2 s.t. every expert's overflow beyond s1 packs
    into <= NCORES chunks of size <= s2."""
    best = None
    for s1 in range(max(cnts) // 2, max(cnts) + 1):
        ov = [max(c - s1, 0) for c in cnts]
        if sum(ov) == 0:
            s2 = 0
        else:
            s2 = None
            for t in range(1, max(ov) + 1):
                if sum(-(-o // t) for o in ov if o) <= NCORES:
                    s2 = t
                    break
            if s2 is None:
                continue
        if best is None or s1 + s2 < best[0] + best[1]:
            best = (s1, s2)
    return best


def _seg1_blocks(s1):
    """Ascending seg1 block list: small startup chunks, then 256s, tail."""
    sizes = [64, 64, 128, 128, 128]
    rem = s1 - 512
    assert rem >= 0
    while rem >= 256:
        sizes.append(256)
        rem -= 256
    if rem:
        sizes.append(rem)
    out = []
    t0 = 0
    for n in sizes:
        out.append((t0, n))
        t0 += n
    return out


def _pb_blocks(s1):
    """Phase B seg1 blocks: ascending 512-chunks."""
    out = []
    t0 = 0
    while t0 < s1:
        n = min(BLK, s1 - t0)
        out.append((t0, n))
        t0 += n
    return out


_CACHE: dict = {}


def _build_bass(s1: int, s2: int):
    """Single SPMD Bass program: seg1 (s1 slots, weight set A) + seg2
    (s2 slots, weight set B)."""
    import concourse.bacc as bacc
    import concourse.mybir as mybir
    import concourse.tile as tile

    f32 = mybir.dt.float32
    bf16 = mybir.dt.bfloat16
    S = s1 + s2
    blocksA = _seg1_blocks(s1)
    blocksPB = _pb_blocks(s1)

    nc = bacc.Bacc("TRN2", target_bir_lowering=False)

    # chunk-major token layout: block (t0, n) occupies cols [DCH*t0, DCH*(t0+n))
    # as [dc, tok]; the seg2 block sits at [DCH*s1, DCH*S)
    xt_d = nc.dram_tensor("xt", [P, DCH * S], bf16, kind="ExternalInput")
    wstA_d = nc.dram_tensor("wstA", [ICH, P, 2 * DCH * P], bf16, kind="ExternalInput")
    w2tA_d = nc.dram_tensor("w2tA", [DCH, P, ICH * P], bf16, kind="ExternalInput")
    if s2:
        wstB_d = nc.dram_tensor("wstB", [ICH, P, 2 * DCH * P], bf16, kind="ExternalInput")
        w2tB_d = nc.dram_tensor("w2tB", [DCH, P, ICH * P], bf16, kind="ExternalInput")
    out_d = nc.dram_tensor("out", [DCH, P, S], bf16, kind="ExternalOutput")

    with tile.TileContext(nc) as tc:
        with (
            tc.tile_pool(name="xpool", bufs=1) as xpool,
            tc.tile_pool(name="hpool", bufs=1) as hpool,
            tc.tile_pool(name="wpool", bufs=3) as wpool,
            tc.tile_pool(name="w2pool", bufs=2) as w2pool,
            tc.tile_pool(name="spool", bufs=6) as spool,
            tc.tile_pool(name="opool", bufs=3) as opool,
            tc.tile_pool(name="ps", bufs=8, space="PSUM") as ps_pool,
        ):
            hT = hpool.tile([P, ICH, S], bf16)

            # startup DMA order: gate A half, up A half, seg1 token chunks
            # smallest-first, then seg2 weights + tokens
            wgA0 = wpool.tile([P, DCH * P], bf16, tag="wgA")
            nc.sync.dma_start(wgA0[:], wstA_d[0][:, : DCH * P])
            wuA0 = wpool.tile([P, DCH * P], bf16, tag="wuA")
            nc.sync.dma_start(wuA0[:], wstA_d[0][:, DCH * P :])
            xb = {}
            for t0, n in blocksA:
                xt = xpool.tile([P, DCH, n], bf16, tag=f"xb{t0}")
                xb[t0] = xt
                nc.sync.dma_start(xt[:], xt_d[:, DCH * t0 : DCH * (t0 + n)])

            # ---- phase A: gate/up + SwiGLU, h resident ----
            segq = []
            def issue_B(icb):
                wgB = wpool.tile([P, DCH * P], bf16, tag="wgB")
                nc.sync.dma_start(wgB[:], wstB_d[icb][:, : DCH * P])
                wuB = wpool.tile([P, DCH * P], bf16, tag="wuB")
                nc.sync.dma_start(wuB[:], wstB_d[icb][:, DCH * P :])
                segq.append((icb, wgB, wuB))
            for ic in range(ICH):
                if ic == 0:
                    wgA, wuA = wgA0, wuA0
                else:
                    wgA = wpool.tile([P, DCH * P], bf16, tag="wgA")
                    nc.sync.dma_start(wgA[:], wstA_d[ic][:, : DCH * P])
                    wuA = wpool.tile([P, DCH * P], bf16, tag="wuA")
                    nc.sync.dma_start(wuA[:], wstA_d[ic][:, DCH * P :])
                if s2 and ic >= 1:
                    issue_B(ic - 1)
                    if ic == 1:
                        xt2 = xpool.tile([P, DCH, s2], bf16, tag="xb2")
                        nc.sync.dma_start(xt2[:], xt_d[:, DCH * s1 :])
                units = [(None, t0, n, wgA, wuA, xb[t0]) for t0, n in blocksA]
                if s2 and ic >= 2:
                    icb, wgB, wuB = segq.pop(0)
                    units.append((icb, s1, s2, wgB, wuB, xt2))
                for icw, t0, n, wg, wu, xt in units:
                    pg = ps_pool.tile([P, BLK], f32, tag="ps", name=f"pg_{ic}_{icw}_{t0}")
                    pu = ps_pool.tile([P, BLK], f32, tag="ps", name=f"pu_{ic}_{icw}_{t0}")
                    for dc in range(DCH):
                        nc.tensor.matmul(
                            pg[:, :n],
                            wg[:, dc * P : (dc + 1) * P],
                            xt[:, dc, :],
                            start=(dc == 0),
                            stop=(dc == DCH - 1),
                        )
                        nc.tensor.matmul(
                            pu[:, :n],
                            wu[:, dc * P : (dc + 1) * P],
                            xt[:, dc, :],
                            start=(dc == 0),
                            stop=(dc == DCH - 1),
                        )
                    sg = spool.tile([P, BLK], bf16, tag="sg")
                    nc.scalar.activation(
                        sg[:, :n], pg[:, :n], mybir.ActivationFunctionType.Silu
                    )
                    ich = ic if icw is None else icw
                    nc.vector.tensor_mul(hT[:, ich, t0 : t0 + n], sg[:, :n], pu[:, :n])

            if s2:
                issue_B(ICH - 1)
                for icw, wgB, wuB in segq:
                    pg = ps_pool.tile([P, BLK], f32, tag="ps", name=f"pgd_{icw}")
                    pu = ps_pool.tile([P, BLK], f32, tag="ps", name=f"pud_{icw}")
                    for dc in range(DCH):
                        nc.tensor.matmul(pg[:, :s2], wgB[:, dc * P : (dc + 1) * P], xt2[:, dc, :], start=(dc == 0), stop=(dc == DCH - 1))
                        nc.tensor.matmul(pu[:, :s2], wuB[:, dc * P : (dc + 1) * P], xt2[:, dc, :], start=(dc == 0), stop=(dc == DCH - 1))
                    sg = spool.tile([P, BLK], bf16, tag="sg")
                    nc.scalar.activation(sg[:, :s2], pg[:, :s2], mybir.ActivationFunctionType.Silu)
                    nc.vector.tensor_mul(hT[:, icw, s1:], sg[:, :s2], pu[:, :s2])

            # ---- phase B: down proj ----
            for dc in range(DCH):
                w2A = w2pool.tile([P, ICH * P], bf16, tag="w2A")
                nc.sync.dma_start(w2A[:], w2tA_d[dc])
                units = [(t0, n, w2A) for t0, n in blocksPB]
                if s2:
                    w2B = w2pool.tile([P, ICH * P], bf16, tag="w2B")
                    nc.sync.dma_start(w2B[:], w2tB_d[dc])
                    units.append((s1, s2, w2B))
                for t0, n, w2 in units:
                    po = ps_pool.tile([P, BLK], f32, tag="ps", name=f"po_{dc}_{t0}")
                    for ic in range(ICH):
                        nc.tensor.matmul(
                            po[:, :n],
                            w2[:, ic * P : (ic + 1) * P],
                            hT[:, ic, t0 : t0 + n],
                            start=(ic == 0),
                            stop=(ic == ICH - 1),
                        )
                    ob = opool.tile([P, BLK], bf16, tag="ob")
                    nc.scalar.activation(
                        ob[:, :n], po[:, :n], mybir.ActivationFunctionType.Copy
                    )
                    nc.sync.dma_start(out_d[dc, :, t0 : t0 + n], ob[:, :n])

    nc.compile()
    return nc


def _prepare(hidden_states, router_w, ws, w2s):
    """Host-side routing, balanced two-segment packing, weight transposes,
    bf16 casts."""
    import ml_dtypes

    bf16 = ml_dtypes.bfloat16

    x = np.asarray(hidden_states, dtype=np.float32).reshape(T, D)
    router_w = np.asarray(router_w, dtype=np.float32)
    ws = np.asarray(ws, dtype=np.float32)
    w2s = np.asarray(w2s, dtype=np.float32)

    top1, top2, w1, w2 = _host_router(x, router_w)

    # per-expert token lists; loc/exp give each token-contribution's position
    toks: list[list[int]] = [[] for _ in range(E)]
    loc = np.zeros((2, T), dtype=np.int64)
    exp = np.zeros((2, T), dtype=np.int64)
    for k, ti in enumerate((top1, top2)):
        for t in range(T):
            e = int(ti[t])
            loc[k, t] = len(toks[e])
            exp[k, t] = e
            toks[e].append(t)
    cnts = [len(tk) for tk in toks]

    s1, s2 = _split_slots(cnts)
    S = s1 + s2

    # overflow chunks of <= s2 tokens, assigned one per core (in core order)
    pieces = []  # (expert, start, end)
    for e in range(E):
        a = s1
        while a < cnts[e]:
            b = min(a + s2, cnts[e])
            pieces.append((e, a, b))
            a = b
    assert len(pieces) <= NCORES, (s1, s2, pieces)
    piece_of_core = {c: pieces[c] for c in range(len(pieces))}

    # slot map: expert e's j-th packed token -> (core, slot)
    slot_core = [np.empty(cnts[e], dtype=np.int64) for e in range(E)]
    slot_idx = [np.empty(cnts[e], dtype=np.int64) for e in range(E)]
    for e in range(E):
        n1 = min(cnts[e], s1)
        slot_core[e][:n1] = e
        slot_idx[e][:n1] = np.arange(n1)
    for c, (e, a, b) in piece_of_core.items():
        slot_core[e][a:b] = c
        slot_idx[e][a:b] = s1 + np.arange(b - a)

    # flat packed position of each token-contribution
    pos = np.empty((2, T), dtype=np.int64)
    for k in range(2):
        for t in range(T):
            e = exp[k, t]
            j = loc[k, t]
            pos[k, t] = slot_core[e][j] * S + slot_idx[e][j]

    x_bf = x.astype(bf16)
    blocksA = _seg1_blocks(s1)

    def chunk_major(tok_list, nslots):
        """[P, DCH*nslots] chunk-major over the given block partition."""
        xe = np.zeros((nslots, DCH, P), dtype=bf16)
        m = len(tok_list)
        if m:
            xe[:m] = x_bf[tok_list].reshape(m, DCH, P)
        return xe

    xt_all = []
    for c in range(NCORES):
        xe1 = chunk_major(toks[c][: min(cnts[c], s1)], s1)
        xtc = np.empty((P, DCH * S), dtype=bf16)
        for t0, n in blocksA:
            xtc[:, DCH * t0 : DCH * (t0 + n)] = (
                xe1[t0 : t0 + n].transpose(2, 1, 0).reshape(P, DCH * n)
            )
        if s2:
            if c in piece_of_core:
                e, a, b = piece_of_core[c]
                xe2 = chunk_major(toks[e][a:b], s2)
            else:
                xe2 = np.zeros((s2, DCH, P), dtype=bf16)
            xtc[:, DCH * s1 :] = xe2.transpose(2, 1, 0).reshape(P, DCH * s2)
        xt_all.append(xtc)

    wst_all = []
    w2t_all = []
    for e in range(E):
        gate = ws[e, :I, :]  # [I, D]
        up = ws[e, I:, :]
        gt = gate.reshape(ICH, P, DCH, P).transpose(0, 3, 2, 1)
        ut = up.reshape(ICH, P, DCH, P).transpose(0, 3, 2, 1)
        wst = np.stack([gt, ut], axis=2)  # [ICH, P, 2, DCH, P]
        wst_all.append(
            np.ascontiguousarray(wst.reshape(ICH, P, 2 * DCH * P).astype(bf16))
        )
        w2t = w2s[e].reshape(DCH, P, ICH, P).transpose(0, 3, 2, 1)
        w2t_all.append(
            np.ascontiguousarray(w2t.reshape(DCH, P, ICH * P).astype(bf16))
        )

    return (s1, s2), piece_of_core, pos, (w1, w2), xt_all, wst_all, w2t_all


def kernel(hidden_states, router_w, ws, w2s):
    from concourse import bass_utils

    hs = np.asarray(hidden_states)
    B, Sq, _ = hs.shape
    (s1, s2), piece_of_core, pos, (w1, w2), xt_all, wst_all, w2t_all = _prepare(
        hidden_states, router_w, ws, w2s
    )
    S = s1 + s2

    if (s1, s2) not in _CACHE:
        _CACHE[(s1, s2)] = _build_bass(s1, s2)
    nc = _CACHE[(s1, s2)]

    in_maps = []
    for c in range(NCORES):
        m = {"xt": xt_all[c], "wstA": wst_all[c], "w2tA": w2t_all[c]}
        if s2:
            eb = piece_of_core[c][0] if c in piece_of_core else c
            m["wstB"] = wst_all[eb]
            m["w2tB"] = w2t_all[eb]
        in_maps.append(m)
    res = bass_utils.run_bass_kernel_spmd(nc, in_maps, core_ids=list(range(NCORES)))

    # assemble: per-core out [DCH, P, S] -> flat packed [NCORES*S, D]
    packed = np.empty((NCORES * S, D), dtype=np.float32)
    for c in range(NCORES):
        oc = np.asarray(res.results[c]["out"]).astype(np.float32)  # [DCH, P, S]
        packed[c * S : (c + 1) * S] = oc.reshape(D, S).T

    out = w1[:, None] * packed[pos[0]] + w2[:, None] * packed[pos[1]]
    return out.reshape(B, Sq, D).astype(np.float32)



# revision 7
# speedup vs baseline: 1.3111x; 1.3111x over previous
"""DBRX MoE experts kernel for Trainium2 (8 NeuronCores).

Strategy (balanced expert-parallel, split-fp8 DoubleRow, zero collectives):
  - Router (logits -> softmax -> top-2 -> renormalize) computed on host in
    numpy (0.01% of FLOPs); it determines the token->expert dispatch.
  - Every core runs the identical SPMD program over S = s1 + s2 token slots,
    two segments with independent weight sets (per-core input data):
      seg1 (s1 slots): core c's own expert c, tokens [0, min(cnt_c, s1)).
      seg2 (s2 slots): an overflow chunk of some (possibly other) expert's
        tokens beyond s1, assigned per-core so the 8 chunks cover all
        overflow. (s1, s2) minimizes s1+s2 subject to the overflow of every
        expert packing into <= 8 chunks of size s2.
  - All matmuls are fp8e4 (e4m3) in DoubleRow perf mode: one instruction
    contracts 2x128 K at 0.5 cycles per moving row (4x bf16 FLOP rate under
    the TRN2 cost model). Accuracy is recovered with a split-precision
    (Ozaki-style) 3-term scheme per 256-wide K-pair:
        w8*x8 + r8*x8 + w8*xr8
    where r8 = q8(w_scaled - w8), xr8 = q8(x_scaled - x8) are fp8 residuals
    quantized at the SAME power-of-2 scale, so all terms accumulate into one
    PSUM group. Net cost 0.75x bf16 for ~bf16 accuracy (rel err ~2e-3).
    A fraction of K-pairs (TIER1_*) drop the two residual terms (pure fp8,
    0.25x cost) to spend the remaining error budget (gate: 2e-2).
  - Scales are global powers of 2 (SX=16 tokens, SW=512 weights, SH=8 for
    the resident h), folded into activation-engine input scales: silu does
    the gate descale, the h8 quantize copy applies SH/(SX*SW), and the final
    PSUM evacuation applies 1/(SH*SW2). h residual hr8 is produced by one
    DVE scalar_tensor_tensor: (h_tmp * SH/(SX*SW)) - h8.
  - Intermediates fp16 (sg, final out) instead of bf16: 8x finer mantissa,
    same bytes. Output rows are the *unweighted* FFN outputs; the host
    scales by the renormalized top-2 weights and sums (fp32).
  - Phase A (gate/up): per 128-row I-chunk, stream the fp8 gate/up weight
    tiles (hi+residual, same bytes as bf16) once; tokens are the moving dim;
    h8/hr8 stay resident in SBUF. Phase B (down): per 128-row D-chunk,
    stream w2 (hi+residual); accumulate all 16 I-pairs into one PSUM bank;
    evacuate fp16 and DMA out.
  - Token DMAs are chunk-major ([dc, tok] contiguous per block); startup
    chunks are small so the first matmul fires early while the PE clock
    ramps.
"""

import numpy as np

T = 4096
D = 2048
E = 8
I = 4096
NCORES = 8
P = 128
DCH = D // P  # 16 d-chunks (8 DoubleRow pairs)
ICH = I // P  # 32 i-chunks (16 DoubleRow pairs)
PA = DCH // 2  # 8 k-pairs in phase A
PB = ICH // 2  # 16 k-pairs in phase B
BLK = 512  # token block (PSUM bank = 512 fp32)

SX = 16.0
SW = 512.0
SH = 8.0
SW2 = 512.0
SILU_SC = 1.0 / (SX * SW)
H8_SC = SH / (SX * SW)
OUT_SC = 1.0 / (SH * SW2)

# fraction of (path, ic, pair) cells computed pure-fp8 (no residual terms)
TIER1_A = 0.0
TIER1_B = 0.0


def _t1(idx: int, frac: float) -> bool:
    """Deterministic pseudo-uniform tier-1 cell selection."""
    return ((idx * 2654435761) & 0xFFFFFFFF) < frac * 4294967296.0


def _host_router(x, router_w):
    """Replicate reference routing in numpy (fp32)."""
    logits = (x.astype(np.float64) @ router_w.astype(np.float64).T).astype(np.float32)
    m = logits.max(axis=-1, keepdims=True)
    ex = np.exp((logits - m).astype(np.float32))
    probs = ex / ex.sum(axis=-1, keepdims=True)
    # top-2, ties to lower index (matches jax.lax.top_k)
    top1 = probs.argmax(axis=-1)
    p = probs.copy()
    p[np.arange(T), top1] = -1.0
    top2 = p.argmax(axis=-1)
    w1 = probs[np.arange(T), top1]
    w2 = probs[np.arange(T), top2]
    s = w1 + w2
    return top1.astype(np.int64), top2.astype(np.int64), (w1 / s).astype(np.float32), (w2 / s).astype(np.float32)


def _split_slots(cnts):
    """Minimize S = s1 + s2 s.t. every expert's overflow beyond s1 packs
    into <= NCORES chunks of size <= s2. Both s1 and s2 are kept multiples
    of 4 (fp8 access patterns need 4-byte-aligned strides)."""
    best = None
    lo = max(512, max(cnts) // 2)
    for s1 in range(-(-lo // 4) * 4, max(cnts) + 4, 4):
        ov = [max(c - s1, 0) for c in cnts]
        if sum(ov) == 0:
            s2 = 0
        else:
            s2 = None
            for t in range(4, max(ov) + 4, 4):
                if sum(-(-o // t) for o in ov if o) <= NCORES:
                    s2 = t
                    break
            if s2 is None:
                continue
        if best is None or s1 + s2 < best[0] + best[1]:
            best = (s1, s2)
    return best


def _seg1_blocks(s1):
    """Ascending seg1 block list: small startup chunks, then 256s, tail."""
    sizes = [64, 64, 128, 128, 128]
    rem = s1 - 512
    assert rem >= 0
    while rem >= 256:
        sizes.append(256)
        rem -= 256
    if rem:
        sizes.append(rem)
    out = []
    t0 = 0
    for n in sizes:
        out.append((t0, n))
        t0 += n
    return out


def _pb_blocks(s1):
    """Phase B seg1 blocks: ascending 512-chunks."""
    out = []
    t0 = 0
    while t0 < s1:
        n = min(BLK, s1 - t0)
        out.append((t0, n))
        t0 += n
    return out


_CACHE: dict = {}


def _build_bass(s1: int, s2: int):
    """Single SPMD Bass program: seg1 (s1 slots, weight set A) + seg2
    (s2 slots, weight set B), split-fp8 DoubleRow matmuls."""
    import concourse.bacc as bacc
    import concourse.mybir as mybir
    import concourse.tile as tile

    f32 = mybir.dt.float32
    fp8 = mybir.dt.float8e4
    fp16 = mybir.dt.float16
    DR = mybir.MatmulPerfMode.DoubleRow
    Silu = mybir.ActivationFunctionType.Silu
    Copy = mybir.ActivationFunctionType.Copy
    S = s1 + s2
    blocksA = _seg1_blocks(s1)
    blocksPB = _pb_blocks(s1)

    nc = bacc.Bacc("TRN2", target_bir_lowering=False)

    # chunk-major token layout: block (t0, n) occupies cols [DCH*t0, DCH*(t0+n))
    # as [dc, tok]; the seg2 block sits at [DCH*s1, DCH*S)
    xt_d = nc.dram_tensor("xt", [P, DCH * S], fp8, kind="ExternalInput")
    xr_d = nc.dram_tensor("xr", [P, DCH * S], fp8, kind="ExternalInput")
    # per ic: two halves (gate, up), each [P, {hi,res}, PA pairs, 2 ktiles, P]
    wstA_d = nc.dram_tensor("wstA", [ICH, 2, P, 2 * PA * 2 * P], fp8, kind="ExternalInput")
    # per dc: [P, {hi,res}, PB pairs, 2 ktiles, P]
    w2tA_d = nc.dram_tensor("w2tA", [DCH, P, 2 * PB * 2 * P], fp8, kind="ExternalInput")
    if s2:
        wstB_d = nc.dram_tensor("wstB", [ICH, 2, P, 2 * PA * 2 * P], fp8, kind="ExternalInput")
        w2tB_d = nc.dram_tensor("w2tB", [DCH, P, 2 * PB * 2 * P], fp8, kind="ExternalInput")
    out_d = nc.dram_tensor("out", [DCH, P, S], fp16, kind="ExternalOutput")

    with tile.TileContext(nc) as tc:
        with (
            tc.tile_pool(name="xpool", bufs=1) as xpool,
            tc.tile_pool(name="hpool", bufs=1) as hpool,
            tc.tile_pool(name="wpool", bufs=3) as wpool,
            tc.tile_pool(name="w2pool", bufs=2) as w2pool,
            tc.tile_pool(name="spool", bufs=4) as spool,
            tc.tile_pool(name="opool", bufs=3) as opool,
            tc.tile_pool(name="ps", bufs=8, space="PSUM") as ps_pool,
        ):
            h8T = hpool.tile([P, ICH, S], fp8)
            hrT = hpool.tile([P, ICH, S], fp8)

            WSHP = [P, 2, PA, 2, P]  # {hi,res}, pair, ktile, m

            # startup DMA order: gate A half, up A half, seg1 token chunks
            # smallest-first, then seg2 weights + tokens
            wgA0 = wpool.tile(WSHP, fp8, tag="wgA")
            nc.sync.dma_start(wgA0[:], wstA_d[0, 0])
            wuA0 = wpool.tile(WSHP, fp8, tag="wuA")
            nc.sync.dma_start(wuA0[:], wstA_d[0, 1])
            xb = {}
            for t0, n in blocksA:
                x8t = xpool.tile([P, DCH, n], fp8, tag=f"x8b{t0}")
                nc.sync.dma_start(x8t[:], xt_d[:, DCH * t0 : DCH * (t0 + n)])
                xrt = xpool.tile([P, DCH, n], fp8, tag=f"xrb{t0}")
                nc.sync.dma_start(xrt[:], xr_d[:, DCH * t0 : DCH * (t0 + n)])
                xb[t0] = (x8t, xrt)

            def unit_A(pg, pu, wg, wu, x8t, xrt, n, ich, t0):
                """One (i-chunk, token-block) phase-A unit."""
                for path, wt, pp in ((0, wg, pg), (1, wu, pu)):
                    for c in range(PA):
                        t3 = not _t1((ich * PA + c) * 2 + path, TIER1_A)
                        nc.tensor.matmul(
                            pp[:, :n], wt[:, 0, c], x8t[:, 2 * c : 2 * c + 2, :],
                            start=(c == 0), stop=(c == PA - 1 and not t3), perf_mode=DR,
                        )
                        if t3:
                            nc.tensor.matmul(
                                pp[:, :n], wt[:, 1, c], x8t[:, 2 * c : 2 * c + 2, :],
                                start=False, stop=False, perf_mode=DR,
                            )
                            nc.tensor.matmul(
                                pp[:, :n], wt[:, 0, c], xrt[:, 2 * c : 2 * c + 2, :],
                                start=False, stop=(c == PA - 1), perf_mode=DR,
                            )
                sg = spool.tile([P, BLK], fp16, tag="sg")
                nc.scalar.activation(sg[:, :n], pg[:, :n], Silu, scale=SILU_SC)
                ht = spool.tile([P, BLK], f32, tag="ht")
                nc.vector.tensor_mul(ht[:, :n], sg[:, :n], pu[:, :n])
                nc.scalar.activation(
                    h8T[:, ich, t0 : t0 + n], ht[:, :n], Copy, scale=H8_SC
                )
                nc.vector.scalar_tensor_tensor(
                    hrT[:, ich, t0 : t0 + n], ht[:, :n], H8_SC, h8T[:, ich, t0 : t0 + n],
                    mybir.AluOpType.mult, mybir.AluOpType.subtract,
                )

            # ---- phase A: gate/up + SwiGLU, h8/hr8 resident ----
            segq = []

            def issue_B(icb):
                wgB = wpool.tile(WSHP, fp8, tag="wgB")
                nc.sync.dma_start(wgB[:], wstB_d[icb, 0])
                wuB = wpool.tile(WSHP, fp8, tag="wuB")
                nc.sync.dma_start(wuB[:], wstB_d[icb, 1])
                segq.append((icb, wgB, wuB))

            for ic in range(ICH):
                if ic == 0:
                    wgA, wuA = wgA0, wuA0
                else:
                    wgA = wpool.tile(WSHP, fp8, tag="wgA")
                    nc.sync.dma_start(wgA[:], wstA_d[ic, 0])
                    wuA = wpool.tile(WSHP, fp8, tag="wuA")
                    nc.sync.dma_start(wuA[:], wstA_d[ic, 1])
                if s2 and ic >= 1:
                    issue_B(ic - 1)
                    if ic == 1:
                        x8t2 = xpool.tile([P, DCH, s2], fp8, tag="x8b2")
                        nc.sync.dma_start(x8t2[:], xt_d[:, DCH * s1 :])
                        xrt2 = xpool.tile([P, DCH, s2], fp8, tag="xrb2")
                        nc.sync.dma_start(xrt2[:], xr_d[:, DCH * s1 :])
                units = [(None, t0, n, wgA, wuA) + xb[t0] for t0, n in blocksA]
                if s2 and ic >= 2:
                    icb, wgB, wuB = segq.pop(0)
                    units.append((icb, s1, s2, wgB, wuB, x8t2, xrt2))
                for icw, t0, n, wg, wu, x8t, xrt in units:
                    ich = ic if icw is None else icw
                    pg = ps_pool.tile([P, BLK], f32, tag="ps", name=f"pg_{ic}_{icw}_{t0}")
                    pu = ps_pool.tile([P, BLK], f32, tag="ps", name=f"pu_{ic}_{icw}_{t0}")
                    unit_A(pg, pu, wg, wu, x8t, xrt, n, ich, t0)

            if s2:
                issue_B(ICH - 1)
                for icw, wgB, wuB in segq:
                    pg = ps_pool.tile([P, BLK], f32, tag="ps", name=f"pgd_{icw}")
                    pu = ps_pool.tile([P, BLK], f32, tag="ps", name=f"pud_{icw}")
                    unit_A(pg, pu, wgB, wuB, x8t2, xrt2, s2, icw, s1)

            # ---- phase B: down proj ----
            W2SHP = [P, 2, PB, 2, P]
            for dc in range(DCH):
                w2A = w2pool.tile(W2SHP, fp8, tag="w2A")
                nc.sync.dma_start(w2A[:], w2tA_d[dc])
                units = [(t0, n, w2A) for t0, n in blocksPB]
                if s2:
                    w2B = w2pool.tile(W2SHP, fp8, tag="w2B")
                    nc.sync.dma_start(w2B[:], w2tB_d[dc])
                    units.append((s1, s2, w2B))
                for t0, n, w2 in units:
                    po = ps_pool.tile([P, BLK], f32, tag="ps", name=f"po_{dc}_{t0}")
                    for c in range(PB):
                        t3 = not _t1(0x10000 + dc * PB + c, TIER1_B)
                        nc.tensor.matmul(
                            po[:, :n], w2[:, 0, c], h8T[:, 2 * c : 2 * c + 2, t0 : t0 + n],
                            start=(c == 0), stop=(c == PB - 1 and not t3), perf_mode=DR,
                        )
                        if t3:
                            nc.tensor.matmul(
                                po[:, :n], w2[:, 1, c], h8T[:, 2 * c : 2 * c + 2, t0 : t0 + n],
                                start=False, stop=False, perf_mode=DR,
                            )
                            nc.tensor.matmul(
                                po[:, :n], w2[:, 0, c], hrT[:, 2 * c : 2 * c + 2, t0 : t0 + n],
                                start=False, stop=(c == PB - 1), perf_mode=DR,
                            )
                    ob = opool.tile([P, BLK], fp16, tag="ob")
                    nc.scalar.activation(ob[:, :n], po[:, :n], Copy, scale=OUT_SC)
                    nc.sync.dma_start(out_d[dc, :, t0 : t0 + n], ob[:, :n])

    nc.compile()
    return nc


def _prepare(hidden_states, router_w, ws, w2s):
    """Host-side routing, balanced two-segment packing, fp8 split
    quantization, weight transposes."""
    import ml_dtypes

    E4 = ml_dtypes.float8_e4m3

    x = np.asarray(hidden_states, dtype=np.float32).reshape(T, D)
    router_w = np.asarray(router_w, dtype=np.float32)
    ws = np.asarray(ws, dtype=np.float32)
    w2s = np.asarray(w2s, dtype=np.float32)

    top1, top2, w1, w2 = _host_router(x, router_w)

    # per-expert token lists; loc/exp give each token-contribution's position
    toks: list[list[int]] = [[] for _ in range(E)]
    loc = np.zeros((2, T), dtype=np.int64)
    exp = np.zeros((2, T), dtype=np.int64)
    for k, ti in enumerate((top1, top2)):
        for t in range(T):
            e = int(ti[t])
            loc[k, t] = len(toks[e])
            exp[k, t] = e
            toks[e].append(t)
    cnts = [len(tk) for tk in toks]

    s1, s2 = _split_slots(cnts)
    S = s1 + s2

    # overflow chunks of <= s2 tokens, assigned one per core (in core order)
    pieces = []  # (expert, start, end)
    for e in range(E):
        a = s1
        while a < cnts[e]:
            b = min(a + s2, cnts[e])
            pieces.append((e, a, b))
            a = b
    assert len(pieces) <= NCORES, (s1, s2, pieces)
    piece_of_core = {c: pieces[c] for c in range(len(pieces))}

    # slot map: expert e's j-th packed token -> (core, slot)
    slot_core = [np.empty(cnts[e], dtype=np.int64) for e in range(E)]
    slot_idx = [np.empty(cnts[e], dtype=np.int64) for e in range(E)]
    for e in range(E):
        n1 = min(cnts[e], s1)
        slot_core[e][:n1] = e
        slot_idx[e][:n1] = np.arange(n1)
    for c, (e, a, b) in piece_of_core.items():
        slot_core[e][a:b] = c
        slot_idx[e][a:b] = s1 + np.arange(b - a)

    # flat packed position of each token-contribution
    pos = np.empty((2, T), dtype=np.int64)
    for k in range(2):
        for t in range(T):
            e = exp[k, t]
            j = loc[k, t]
            pos[k, t] = slot_core[e][j] * S + slot_idx[e][j]

    x8 = (x * SX).astype(E4)
    xr8 = (x * SX - x8.astype(np.float32)).astype(E4)
    blocksA = _seg1_blocks(s1)

    def chunk_major(src, tok_list, nslots):
        """[nslots, DCH, P] from the given token rows of src."""
        xe = np.zeros((nslots, DCH, P), dtype=E4)
        m = len(tok_list)
        if m:
            xe[:m] = src[tok_list].reshape(m, DCH, P)
        return xe

    def pack_tokens(src, c):
        xe1 = chunk_major(src, toks[c][: min(cnts[c], s1)], s1)
        xtc = np.empty((P, DCH * S), dtype=E4)
        for t0, n in blocksA:
            xtc[:, DCH * t0 : DCH * (t0 + n)] = (
                xe1[t0 : t0 + n].transpose(2, 1, 0).reshape(P, DCH * n)
            )
        if s2:
            if c in piece_of_core:
                e, a, b = piece_of_core[c]
                xe2 = chunk_major(src, toks[e][a:b], s2)
            else:
                xe2 = np.zeros((s2, DCH, P), dtype=E4)
            xtc[:, DCH * s1 :] = xe2.transpose(2, 1, 0).reshape(P, DCH * s2)
        return xtc

    xt_all = [pack_tokens(x8, c) for c in range(NCORES)]
    xr_all = [pack_tokens(xr8, c) for c in range(NCORES)]

    def prep_A(W):
        """[I, D] fp32 -> [ICH, 2(g/u half split by caller), ...] see below.
        Returns hi, res in layout [ICH, P(p), PA, 2, P(m)]."""
        Ws = W * SW
        hi = Ws.astype(E4)
        res = (Ws - hi.astype(np.float32)).astype(E4)

        def lay(A):
            # [I, D] -> [ICH, m, PA, 2, p] -> [ICH, p, PA, 2, m]
            return np.ascontiguousarray(
                A.reshape(ICH, P, PA, 2, P).transpose(0, 4, 2, 3, 1)
            )

        return lay(hi), lay(res)

    wst_all = []
    w2t_all = []
    for e in range(E):
        ghi, gres = prep_A(ws[e, :I, :])
        uhi, ures = prep_A(ws[e, I:, :])
        # [ICH, 2(g/u), P, 2(hi/res), PA, 2, P]
        wst = np.empty((ICH, 2, P, 2, PA, 2, P), dtype=E4)
        wst[:, 0, :, 0] = ghi
        wst[:, 0, :, 1] = gres
        wst[:, 1, :, 0] = uhi
        wst[:, 1, :, 1] = ures
        wst_all.append(wst.reshape(ICH, 2, P, 2 * PA * 2 * P))

        W2s = w2s[e] * SW2
        hi2 = W2s.astype(E4)
        res2 = (W2s - hi2.astype(np.float32)).astype(E4)

        def lay2(A):
            # [D, I] -> [DCH, m, PB, 2, p] -> [DCH, p, PB, 2, m]
            return np.ascontiguousarray(
                A.reshape(DCH, P, PB, 2, P).transpose(0, 4, 2, 3, 1)
            )

        w2t = np.empty((DCH, P, 2, PB, 2, P), dtype=E4)
        w2t[:, :, 0] = lay2(hi2)
        w2t[:, :, 1] = lay2(res2)
        w2t_all.append(w2t.reshape(DCH, P, 2 * PB * 2 * P))

    return (s1, s2), piece_of_core, pos, (w1, w2), (xt_all, xr_all), wst_all, w2t_all


def kernel(hidden_states, router_w, ws, w2s):
    from concourse import bass_utils

    hs = np.asarray(hidden_states)
    B, Sq, _ = hs.shape
    (s1, s2), piece_of_core, pos, (w1, w2), (xt_all, xr_all), wst_all, w2t_all = _prepare(
        hidden_states, router_w, ws, w2s
    )
    S = s1 + s2

    if (s1, s2) not in _CACHE:
        _CACHE[(s1, s2)] = _build_bass(s1, s2)
    nc = _CACHE[(s1, s2)]

    in_maps = []
    for c in range(NCORES):
        m = {"xt": xt_all[c], "xr": xr_all[c], "wstA": wst_all[c], "w2tA": w2t_all[c]}
        if s2:
            eb = piece_of_core[c][0] if c in piece_of_core else c
            m["wstB"] = wst_all[eb]
            m["w2tB"] = w2t_all[eb]
        in_maps.append(m)
    res = bass_utils.run_bass_kernel_spmd(nc, in_maps, core_ids=list(range(NCORES)))

    # assemble: per-core out [DCH, P, S] -> flat packed [NCORES*S, D]
    packed = np.empty((NCORES * S, D), dtype=np.float32)
    for c in range(NCORES):
        oc = np.asarray(res.results[c]["out"]).astype(np.float32)  # [DCH, P, S]
        packed[c * S : (c + 1) * S] = oc.reshape(D, S).T

    out = w1[:, None] * packed[pos[0]] + w2[:, None] * packed[pos[1]]
    return out.reshape(B, Sq, D).astype(np.float32)


# revision 8
# speedup vs baseline: 1.3774x; 1.0505x over previous
"""DBRX MoE experts kernel for Trainium2 (8 NeuronCores).

Strategy (balanced expert-parallel, split-fp8 DoubleRow, zero collectives):
  - Router (logits -> softmax -> top-2 -> renormalize) computed on host in
    numpy (0.01% of FLOPs); it determines the token->expert dispatch.
  - Every core runs the identical SPMD program over S = s1 + s2 token slots,
    two segments with independent weight sets (per-core input data):
      seg1 (s1 slots): core c's own expert c, tokens [0, min(cnt_c, s1)).
      seg2 (s2 slots): an overflow chunk of some (possibly other) expert's
        tokens beyond s1, assigned per-core so the 8 chunks cover all
        overflow. (s1, s2) minimizes s1+s2 subject to the overflow of every
        expert packing into <= 8 chunks of size s2.
  - All matmuls are fp8e4 (e4m3) in DoubleRow perf mode: one instruction
    contracts 2x128 K at 0.5 cycles per moving row (4x bf16 FLOP rate under
    the TRN2 cost model). Accuracy is recovered with a split-precision
    (Ozaki-style) 3-term scheme per 256-wide K-pair:
        w8*x8 + r8*x8 + w8*xr8
    where r8 = q8(w_scaled - w8), xr8 = q8(x_scaled - x8) are fp8 residuals
    quantized at the SAME power-of-2 scale, so all terms accumulate into one
    PSUM group. Net cost 0.75x bf16 for ~bf16 accuracy (rel err ~2e-3).
    A fraction of K-pairs (TIER1_*) drop the two residual terms (pure fp8,
    0.25x cost) to spend the remaining error budget (gate: 2e-2).
  - Scales are global powers of 2 (SX=16 tokens, SW=512 weights, SH=8 for
    the resident h), folded into activation-engine input scales: silu does
    the gate descale, the h8 quantize copy applies SH/(SX*SW), and the final
    PSUM evacuation applies 1/(SH*SW2). h residual hr8 is produced by one
    DVE scalar_tensor_tensor: (h_tmp * SH/(SX*SW)) - h8.
  - Intermediates fp16 (sg, final out) instead of bf16: 8x finer mantissa,
    same bytes. Output rows are the *unweighted* FFN outputs; the host
    scales by the renormalized top-2 weights and sums (fp32).
  - Phase A (gate/up): per 128-row I-chunk, stream the fp8 gate/up weight
    tiles (hi+residual, same bytes as bf16) once; tokens are the moving dim;
    h8/hr8 stay resident in SBUF. Phase B (down): per 128-row D-chunk,
    stream w2 (hi+residual); accumulate all 16 I-pairs into one PSUM bank;
    evacuate fp16 and DMA out.
  - Token DMAs are chunk-major ([dc, tok] contiguous per block); startup
    chunks are small so the first matmul fires early while the PE clock
    ramps.
"""

import numpy as np

T = 4096
D = 2048
E = 8
I = 4096
NCORES = 8
P = 128
DCH = D // P  # 16 d-chunks (8 DoubleRow pairs)
ICH = I // P  # 32 i-chunks (16 DoubleRow pairs)
PA = DCH // 2  # 8 k-pairs in phase A
PB = ICH // 2  # 16 k-pairs in phase B
BLK = 512  # token block (PSUM bank = 512 fp32)

SX = 16.0
SW = 512.0
SH = 8.0
SW2 = 512.0
SILU_SC = 1.0 / (SX * SW)
H8_SC = SH / (SX * SW)
OUT_SC = 1.0 / (SH * SW2)

# fraction of (path, ic, pair) cells computed pure-fp8 (no residual terms);
# calibrated so total rel err stays ~1.7e-2 (gate 2e-2)
TIER1_A = 0.05
TIER1_B = 0.10


def _t1(idx: int, frac: float) -> bool:
    """Deterministic pseudo-uniform tier-1 cell selection."""
    return ((idx * 2654435761) & 0xFFFFFFFF) < frac * 4294967296.0


def _host_router(x, router_w):
    """Replicate reference routing in numpy (fp32)."""
    logits = (x.astype(np.float64) @ router_w.astype(np.float64).T).astype(np.float32)
    m = logits.max(axis=-1, keepdims=True)
    ex = np.exp((logits - m).astype(np.float32))
    probs = ex / ex.sum(axis=-1, keepdims=True)
    # top-2, ties to lower index (matches jax.lax.top_k)
    top1 = probs.argmax(axis=-1)
    p = probs.copy()
    p[np.arange(T), top1] = -1.0
    top2 = p.argmax(axis=-1)
    w1 = probs[np.arange(T), top1]
    w2 = probs[np.arange(T), top2]
    s = w1 + w2
    return top1.astype(np.int64), top2.astype(np.int64), (w1 / s).astype(np.float32), (w2 / s).astype(np.float32)


def _split_slots(cnts):
    """Minimize S = s1 + s2 s.t. every expert's overflow beyond s1 packs
    into <= NCORES chunks of size <= s2. Both s1 and s2 are kept multiples
    of 4 (fp8 access patterns need 4-byte-aligned strides)."""
    best = None
    lo = max(512, max(cnts) // 2)
    for s1 in range(-(-lo // 4) * 4, max(cnts) + 4, 4):
        ov = [max(c - s1, 0) for c in cnts]
        if sum(ov) == 0:
            s2 = 0
        else:
            s2 = None
            for t in range(4, max(ov) + 4, 4):
                if sum(-(-o // t) for o in ov if o) <= NCORES:
                    s2 = t
                    break
            if s2 is None:
                continue
        if best is None or s1 + s2 < best[0] + best[1]:
            best = (s1, s2)
    return best


def _seg1_blocks(s1):
    """Ascending seg1 block list: small startup chunks, then 256s, tail."""
    sizes = [64, 64, 128, 128, 128]
    rem = s1 - 512
    assert rem >= 0
    while rem >= 256:
        sizes.append(256)
        rem -= 256
    if rem:
        sizes.append(rem)
    out = []
    t0 = 0
    for n in sizes:
        out.append((t0, n))
        t0 += n
    return out


def _pb_blocks(s1):
    """Phase B seg1 blocks: ascending 512-chunks."""
    out = []
    t0 = 0
    while t0 < s1:
        n = min(BLK, s1 - t0)
        out.append((t0, n))
        t0 += n
    return out


_CACHE: dict = {}


def _build_bass(s1: int, s2: int):
    """Single SPMD Bass program: seg1 (s1 slots, weight set A) + seg2
    (s2 slots, weight set B), split-fp8 DoubleRow matmuls."""
    import concourse.bacc as bacc
    import concourse.mybir as mybir
    import concourse.tile as tile

    f32 = mybir.dt.float32
    fp8 = mybir.dt.float8e4
    fp16 = mybir.dt.float16
    DR = mybir.MatmulPerfMode.DoubleRow
    Silu = mybir.ActivationFunctionType.Silu
    Copy = mybir.ActivationFunctionType.Copy
    S = s1 + s2
    blocksA = _seg1_blocks(s1)
    blocksPB = _pb_blocks(s1)

    nc = bacc.Bacc("TRN2", target_bir_lowering=False)

    # chunk-major token layout: block (t0, n) occupies cols [DCH*t0, DCH*(t0+n))
    # as [dc, tok]; the seg2 block sits at [DCH*s1, DCH*S)
    xt_d = nc.dram_tensor("xt", [P, DCH * S], fp8, kind="ExternalInput")
    xr_d = nc.dram_tensor("xr", [P, DCH * S], fp8, kind="ExternalInput")
    # per ic: two halves (gate, up), each [P, {hi,res}, PA pairs, 2 ktiles, P]
    wstA_d = nc.dram_tensor("wstA", [ICH, 2, P, 2 * PA * 2 * P], fp8, kind="ExternalInput")
    # per dc: [P, {hi,res}, PB pairs, 2 ktiles, P]
    w2tA_d = nc.dram_tensor("w2tA", [DCH, P, 2 * PB * 2 * P], fp8, kind="ExternalInput")
    if s2:
        wstB_d = nc.dram_tensor("wstB", [ICH, 2, P, 2 * PA * 2 * P], fp8, kind="ExternalInput")
        w2tB_d = nc.dram_tensor("w2tB", [DCH, P, 2 * PB * 2 * P], fp8, kind="ExternalInput")
    out_d = nc.dram_tensor("out", [DCH, P, S], fp16, kind="ExternalOutput")

    with tile.TileContext(nc) as tc:
        with (
            tc.tile_pool(name="xpool", bufs=1) as xpool,
            tc.tile_pool(name="hpool", bufs=1) as hpool,
            tc.tile_pool(name="wpool", bufs=3) as wpool,
            tc.tile_pool(name="w2pool", bufs=2) as w2pool,
            tc.tile_pool(name="spool", bufs=4) as spool,
            tc.tile_pool(name="opool", bufs=3) as opool,
            tc.tile_pool(name="ps", bufs=8, space="PSUM") as ps_pool,
        ):
            h8T = hpool.tile([P, ICH, S], fp8)
            hrT = hpool.tile([P, ICH, S], fp8)

            WSHP = [P, 2, PA, 2, P]  # {hi,res}, pair, ktile, m

            # startup DMA order: gate A half, up A half, seg1 token chunks
            # smallest-first, then seg2 weights + tokens
            wgA0 = wpool.tile(WSHP, fp8, tag="wgA")
            nc.sync.dma_start(wgA0[:], wstA_d[0, 0])
            wuA0 = wpool.tile(WSHP, fp8, tag="wuA")
            nc.sync.dma_start(wuA0[:], wstA_d[0, 1])
            xb = {}
            for t0, n in blocksA:
                x8t = xpool.tile([P, DCH, n], fp8, tag=f"x8b{t0}")
                nc.sync.dma_start(x8t[:], xt_d[:, DCH * t0 : DCH * (t0 + n)])
                xrt = xpool.tile([P, DCH, n], fp8, tag=f"xrb{t0}")
                nc.sync.dma_start(xrt[:], xr_d[:, DCH * t0 : DCH * (t0 + n)])
                xb[t0] = (x8t, xrt)

            def unit_A(pg, pu, wg, wu, x8t, xrt, n, ich, t0):
                """One (i-chunk, token-block) phase-A unit."""
                for path, wt, pp in ((0, wg, pg), (1, wu, pu)):
                    for c in range(PA):
                        t3 = not _t1((ich * PA + c) * 2 + path, TIER1_A)
                        nc.tensor.matmul(
                            pp[:, :n], wt[:, 0, c], x8t[:, 2 * c : 2 * c + 2, :],
                            start=(c == 0), stop=(c == PA - 1 and not t3), perf_mode=DR,
                        )
                        if t3:
                            nc.tensor.matmul(
                                pp[:, :n], wt[:, 1, c], x8t[:, 2 * c : 2 * c + 2, :],
                                start=False, stop=False, perf_mode=DR,
                            )
                            nc.tensor.matmul(
                                pp[:, :n], wt[:, 0, c], xrt[:, 2 * c : 2 * c + 2, :],
                                start=False, stop=(c == PA - 1), perf_mode=DR,
                            )
                sg = spool.tile([P, BLK], fp16, tag="sg")
                nc.scalar.activation(sg[:, :n], pg[:, :n], Silu, scale=SILU_SC)
                ht = spool.tile([P, BLK], f32, tag="ht")
                nc.vector.tensor_mul(ht[:, :n], sg[:, :n], pu[:, :n])
                nc.scalar.activation(
                    h8T[:, ich, t0 : t0 + n], ht[:, :n], Copy, scale=H8_SC
                )
                nc.vector.scalar_tensor_tensor(
                    hrT[:, ich, t0 : t0 + n], ht[:, :n], H8_SC, h8T[:, ich, t0 : t0 + n],
                    mybir.AluOpType.mult, mybir.AluOpType.subtract,
                )

            # ---- phase A: gate/up + SwiGLU, h8/hr8 resident ----
            segq = []

            def issue_B(icb):
                wgB = wpool.tile(WSHP, fp8, tag="wgB")
                nc.sync.dma_start(wgB[:], wstB_d[icb, 0])
                wuB = wpool.tile(WSHP, fp8, tag="wuB")
                nc.sync.dma_start(wuB[:], wstB_d[icb, 1])
                segq.append((icb, wgB, wuB))

            for ic in range(ICH):
                if ic == 0:
                    wgA, wuA = wgA0, wuA0
                else:
                    wgA = wpool.tile(WSHP, fp8, tag="wgA")
                    nc.sync.dma_start(wgA[:], wstA_d[ic, 0])
                    wuA = wpool.tile(WSHP, fp8, tag="wuA")
                    nc.sync.dma_start(wuA[:], wstA_d[ic, 1])
                if s2 and ic >= 1:
                    issue_B(ic - 1)
                    if ic == 1:
                        x8t2 = xpool.tile([P, DCH, s2], fp8, tag="x8b2")
                        nc.sync.dma_start(x8t2[:], xt_d[:, DCH * s1 :])
                        xrt2 = xpool.tile([P, DCH, s2], fp8, tag="xrb2")
                        nc.sync.dma_start(xrt2[:], xr_d[:, DCH * s1 :])
                units = [(None, t0, n, wgA, wuA) + xb[t0] for t0, n in blocksA]
                if s2 and ic >= 2:
                    icb, wgB, wuB = segq.pop(0)
                    units.append((icb, s1, s2, wgB, wuB, x8t2, xrt2))
                for icw, t0, n, wg, wu, x8t, xrt in units:
                    ich = ic if icw is None else icw
                    pg = ps_pool.tile([P, BLK], f32, tag="ps", name=f"pg_{ic}_{icw}_{t0}")
                    pu = ps_pool.tile([P, BLK], f32, tag="ps", name=f"pu_{ic}_{icw}_{t0}")
                    unit_A(pg, pu, wg, wu, x8t, xrt, n, ich, t0)

            if s2:
                issue_B(ICH - 1)
                for icw, wgB, wuB in segq:
                    pg = ps_pool.tile([P, BLK], f32, tag="ps", name=f"pgd_{icw}")
                    pu = ps_pool.tile([P, BLK], f32, tag="ps", name=f"pud_{icw}")
                    unit_A(pg, pu, wgB, wuB, x8t2, xrt2, s2, icw, s1)

            # ---- phase B: down proj ----
            W2SHP = [P, 2, PB, 2, P]
            for dc in range(DCH):
                w2A = w2pool.tile(W2SHP, fp8, tag="w2A")
                nc.sync.dma_start(w2A[:], w2tA_d[dc])
                units = [(t0, n, w2A) for t0, n in blocksPB]
                if s2:
                    w2B = w2pool.tile(W2SHP, fp8, tag="w2B")
                    nc.sync.dma_start(w2B[:], w2tB_d[dc])
                    units.append((s1, s2, w2B))
                for t0, n, w2 in units:
                    po = ps_pool.tile([P, BLK], f32, tag="ps", name=f"po_{dc}_{t0}")
                    for c in range(PB):
                        t3 = not _t1(0x10000 + dc * PB + c, TIER1_B)
                        nc.tensor.matmul(
                            po[:, :n], w2[:, 0, c], h8T[:, 2 * c : 2 * c + 2, t0 : t0 + n],
                            start=(c == 0), stop=(c == PB - 1 and not t3), perf_mode=DR,
                        )
                        if t3:
                            nc.tensor.matmul(
                                po[:, :n], w2[:, 1, c], h8T[:, 2 * c : 2 * c + 2, t0 : t0 + n],
                                start=False, stop=False, perf_mode=DR,
                            )
                            nc.tensor.matmul(
                                po[:, :n], w2[:, 0, c], hrT[:, 2 * c : 2 * c + 2, t0 : t0 + n],
                                start=False, stop=(c == PB - 1), perf_mode=DR,
                            )
                    ob = opool.tile([P, BLK], fp16, tag="ob")
                    nc.scalar.activation(ob[:, :n], po[:, :n], Copy, scale=OUT_SC)
                    nc.sync.dma_start(out_d[dc, :, t0 : t0 + n], ob[:, :n])

    nc.compile()
    return nc


def _prepare(hidden_states, router_w, ws, w2s):
    """Host-side routing, balanced two-segment packing, fp8 split
    quantization, weight transposes."""
    import ml_dtypes

    E4 = ml_dtypes.float8_e4m3

    x = np.asarray(hidden_states, dtype=np.float32).reshape(T, D)
    router_w = np.asarray(router_w, dtype=np.float32)
    ws = np.asarray(ws, dtype=np.float32)
    w2s = np.asarray(w2s, dtype=np.float32)

    top1, top2, w1, w2 = _host_router(x, router_w)

    # per-expert token lists; loc/exp give each token-contribution's position
    toks: list[list[int]] = [[] for _ in range(E)]
    loc = np.zeros((2, T), dtype=np.int64)
    exp = np.zeros((2, T), dtype=np.int64)
    for k, ti in enumerate((top1, top2)):
        for t in range(T):
            e = int(ti[t])
            loc[k, t] = len(toks[e])
            exp[k, t] = e
            toks[e].append(t)
    cnts = [len(tk) for tk in toks]

    s1, s2 = _split_slots(cnts)
    S = s1 + s2

    # overflow chunks of <= s2 tokens, assigned one per core (in core order)
    pieces = []  # (expert, start, end)
    for e in range(E):
        a = s1
        while a < cnts[e]:
            b = min(a + s2, cnts[e])
            pieces.append((e, a, b))
            a = b
    assert len(pieces) <= NCORES, (s1, s2, pieces)
    piece_of_core = {c: pieces[c] for c in range(len(pieces))}

    # slot map: expert e's j-th packed token -> (core, slot)
    slot_core = [np.empty(cnts[e], dtype=np.int64) for e in range(E)]
    slot_idx = [np.empty(cnts[e], dtype=np.int64) for e in range(E)]
    for e in range(E):
        n1 = min(cnts[e], s1)
        slot_core[e][:n1] = e
        slot_idx[e][:n1] = np.arange(n1)
    for c, (e, a, b) in piece_of_core.items():
        slot_core[e][a:b] = c
        slot_idx[e][a:b] = s1 + np.arange(b - a)

    # flat packed position of each token-contribution
    pos = np.empty((2, T), dtype=np.int64)
    for k in range(2):
        for t in range(T):
            e = exp[k, t]
            j = loc[k, t]
            pos[k, t] = slot_core[e][j] * S + slot_idx[e][j]

    x8 = (x * SX).astype(E4)
    xr8 = (x * SX - x8.astype(np.float32)).astype(E4)
    blocksA = _seg1_blocks(s1)

    def chunk_major(src, tok_list, nslots):
        """[nslots, DCH, P] from the given token rows of src."""
        xe = np.zeros((nslots, DCH, P), dtype=E4)
        m = len(tok_list)
        if m:
            xe[:m] = src[tok_list].reshape(m, DCH, P)
        return xe

    def pack_tokens(src, c):
        xe1 = chunk_major(src, toks[c][: min(cnts[c], s1)], s1)
        xtc = np.empty((P, DCH * S), dtype=E4)
        for t0, n in blocksA:
            xtc[:, DCH * t0 : DCH * (t0 + n)] = (
                xe1[t0 : t0 + n].transpose(2, 1, 0).reshape(P, DCH * n)
            )
        if s2:
            if c in piece_of_core:
                e, a, b = piece_of_core[c]
                xe2 = chunk_major(src, toks[e][a:b], s2)
            else:
                xe2 = np.zeros((s2, DCH, P), dtype=E4)
            xtc[:, DCH * s1 :] = xe2.transpose(2, 1, 0).reshape(P, DCH * s2)
        return xtc

    xt_all = [pack_tokens(x8, c) for c in range(NCORES)]
    xr_all = [pack_tokens(xr8, c) for c in range(NCORES)]

    def prep_A(W):
        """[I, D] fp32 -> [ICH, 2(g/u half split by caller), ...] see below.
        Returns hi, res in layout [ICH, P(p), PA, 2, P(m)]."""
        Ws = W * SW
        hi = Ws.astype(E4)
        res = (Ws - hi.astype(np.float32)).astype(E4)

        def lay(A):
            # [I, D] -> [ICH, m, PA, 2, p] -> [ICH, p, PA, 2, m]
            return np.ascontiguousarray(
                A.reshape(ICH, P, PA, 2, P).transpose(0, 4, 2, 3, 1)
            )

        return lay(hi), lay(res)

    wst_all = []
    w2t_all = []
    for e in range(E):
        ghi, gres = prep_A(ws[e, :I, :])
        uhi, ures = prep_A(ws[e, I:, :])
        # [ICH, 2(g/u), P, 2(hi/res), PA, 2, P]
        wst = np.empty((ICH, 2, P, 2, PA, 2, P), dtype=E4)
        wst[:, 0, :, 0] = ghi
        wst[:, 0, :, 1] = gres
        wst[:, 1, :, 0] = uhi
        wst[:, 1, :, 1] = ures
        wst_all.append(wst.reshape(ICH, 2, P, 2 * PA * 2 * P))

        W2s = w2s[e] * SW2
        hi2 = W2s.astype(E4)
        res2 = (W2s - hi2.astype(np.float32)).astype(E4)

        def lay2(A):
            # [D, I] -> [DCH, m, PB, 2, p] -> [DCH, p, PB, 2, m]
            return np.ascontiguousarray(
                A.reshape(DCH, P, PB, 2, P).transpose(0, 4, 2, 3, 1)
            )

        w2t = np.empty((DCH, P, 2, PB, 2, P), dtype=E4)
        w2t[:, :, 0] = lay2(hi2)
        w2t[:, :, 1] = lay2(res2)
        w2t_all.append(w2t.reshape(DCH, P, 2 * PB * 2 * P))

    return (s1, s2), piece_of_core, pos, (w1, w2), (xt_all, xr_all), wst_all, w2t_all


def kernel(hidden_states, router_w, ws, w2s):
    from concourse import bass_utils

    hs = np.asarray(hidden_states)
    B, Sq, _ = hs.shape
    (s1, s2), piece_of_core, pos, (w1, w2), (xt_all, xr_all), wst_all, w2t_all = _prepare(
        hidden_states, router_w, ws, w2s
    )
    S = s1 + s2

    if (s1, s2) not in _CACHE:
        _CACHE[(s1, s2)] = _build_bass(s1, s2)
    nc = _CACHE[(s1, s2)]

    in_maps = []
    for c in range(NCORES):
        m = {"xt": xt_all[c], "xr": xr_all[c], "wstA": wst_all[c], "w2tA": w2t_all[c]}
        if s2:
            eb = piece_of_core[c][0] if c in piece_of_core else c
            m["wstB"] = wst_all[eb]
            m["w2tB"] = w2t_all[eb]
        in_maps.append(m)
    res = bass_utils.run_bass_kernel_spmd(nc, in_maps, core_ids=list(range(NCORES)))

    # assemble: per-core out [DCH, P, S] -> flat packed [NCORES*S, D]
    packed = np.empty((NCORES * S, D), dtype=np.float32)
    for c in range(NCORES):
        oc = np.asarray(res.results[c]["out"]).astype(np.float32)  # [DCH, P, S]
        packed[c * S : (c + 1) * S] = oc.reshape(D, S).T

    out = w1[:, None] * packed[pos[0]] + w2[:, None] * packed[pos[1]]
    return out.reshape(B, Sq, D).astype(np.float32)
